# revision 19
# baseline (speedup 1.0000x reference)
"""Channel self-attention kernel for TRN2, data-parallel over batch on 8 cores.

Math per batch element (N=4096 tokens, C=64 channels):
    q = x.reshape(N, C);  S = q @ q.T  (symmetric)
    attn = softmax(S, axis=-1);  out = gamma * (attn @ q) + x

Implementation notes (v2, fp8 pipeline):
  - The S matmul computes Y = A*(S - 2t_n) + B directly: lhsT = [qT; 1]
    (65 x 128 fp16), rhs = [A*qT; B - A*|q|^2] (65 x 512 fp16), where
    A = 8/ln2 and B = 96.  Y is the exp argument expressed in fp8e4m3
    *bit units*: exp(S - 2t_n) * 2^{(B-56)/8} == e4m3_from_bits(Y).
    The per-column shift -2t_n keeps all exponents <= ~0 (Cauchy-Schwarz:
    S_mn - 2t_n = q_m.q_n - |q_n|^2), so Z fits fp8 range; any error in
    the shift row is column-uniform and cancels in the softmax divide.
  - Z tiles are produced in fp8e4m3 two ways, split across engines:
      ACT: activation(Exp, scale=1/A, bias=-56.5/A) -> true exp, fp8 out.
      DVE: tensor_scalar(max 0, min 118.49) -> int8 = Schraudolph bit-trick
           (linear-in-mantissa exp approx, ~4% on off-diagonal weights).
    Both land on the same 2^{(B-56)/8} scale (the 0.5 in the ACT bias
    compensates int8 truncation); tiles can be mixed freely per column.
  - The attend matmul runs in fp8 DoubleRow mode: two 128-key chunks per
    pass (contraction 256), 0.5 cycles/col on the PE.  lhsT = vhat8
    [128, 2, 65] = e4m3(gamma*q) plus an exact-1.0 denominator row; rhs =
    Z8 [128, 2, supw].
  - fp8 quantization of gamma*q is corrected exactly on the diagonal:
    out = G/d + ((1+gamma)*q - fp32(vhat8)), which equals
    sum_m (Z_mn/d)(v8[m]-v8[n]) + (1+gamma)q -- fp8 noise only multiplies
    off-diagonal attention mass (~1e-4 relative overall).
  - Matmuls for S run in fp16 at 1 cycle/row; prologue is pipelined in
    groups of 4 row-chunks as before.
"""
import sys
if "/opt/trn_rl_repo" not in sys.path:
    sys.path.insert(0, "/opt/trn_rl_repo")

from contextlib import ExitStack
import math

import numpy as np

import concourse.bass as bass
import concourse.mybir as mybir
import concourse.tile as tile
from concourse import bacc
from concourse.masks import make_identity

P = 128          # partitions
C = 64           # channels (head dim)
B = 8            # batch = number of cores

dt = mybir.dt
AF = mybir.ActivationFunctionType
ALU = mybir.AluOpType
PM = mybir.MatmulPerfMode

AS = 8.0 / math.log(2.0)     # Schraudolph slope: e4m3 bits per e-fold
BSH = 96.0                   # bit offset in the shift row (e4m3 2^5)
EXP_SCALE = 1.0 / AS
EXP_BIAS = -56.5 / AS        # 56 = e4m3 bias<<3; +0.5 matches int8 trunc
CLIP_HI = 118.49             # max bits kept finite in both e4m3 flavors

LDW_OPT = False  # ldw-opt incompatible with explicit 16-bit LDWEIGHTS


def _patch_ldw_opt():
    import concourse.bass_utils as bu
    if getattr(bu, "_ldw_opt_patch", False):
        return
    orig = bu.bir_verify_and_optimise

    def patched(*a, **kw):
        orig_run = bu.run_command

        def run2(argv, **k):
            argv = ["--enable-ldw-opt=true" if x == "--enable-ldw-opt=false" else x
                    for x in argv]
            return orig_run(argv, **k)

        bu.run_command = run2
        try:
            return orig(*a, **kw)
        finally:
            bu.run_command = orig_run

    bu.bir_verify_and_optimise = patched
    bu._ldw_opt_patch = True


def build(ntok=4096, supw=512, z_bufs=6, s_bufs=3, pgrp=4, dve_js0=0, dve_js=1, glag=3):
    """Build the per-core Bass module.

    dve_js0/dve_js: of the 2 chunks in each key pair, how many go to the
    DVE Schraudolph path during super 0 / later supers (0..2)."""
    nch = ntok // P           # query/key chunks of 128
    npair = nch // 2          # fp8 DoubleRow key-chunk pairs
    nsup = ntok // supw       # outer n-blocks
    mw = min(512, supw)       # matmul moving width
    nmm = supw // mw          # matmuls per n-super
    ech = supw // P           # epilogue 128-chunks per n-super
    pgrp = min(pgrp, nch)     # prologue chunks per group
    ngrp = nch // pgrp

    nc = bacc.Bacc("TRN2", target_bir_lowering=False, debug=False,
                   enable_asserts=False)
    x = nc.dram_tensor("x", [ntok, C], dt.float32, kind="ExternalInput")
    g = nc.dram_tensor("gamma", [1], dt.float32, kind="ExternalInput")
    o = nc.dram_tensor("out", [ntok, C], dt.float32, kind="ExternalOutput")

    with tile.TileContext(nc) as tc, ExitStack() as ctx:
        sing = ctx.enter_context(tc.tile_pool(name="sing", bufs=1))

        ident = sing.tile([P, P], dt.float32)
        make_identity(nc, ident)
        gam = sing.tile([P, 1], dt.float32)
        nc.sync.dma_start(out=gam, in_=g.ap().to_broadcast((P, 1)))
        gam1 = sing.tile([P, 1], dt.float32)
        nc.vector.tensor_scalar_add(gam1, gam, 1.0)
        ebias = sing.tile([P, 1], dt.float32)
        nc.gpsimd.memset(ebias, EXP_BIAS)

        # q_sb[p, k, 0:64] = x[token 32p+k, :]; col 64 = B/A - |q|^2
        q_sb = sing.tile([P, nch, C + 1], dt.float32)
        # vhat8[p, k, 0:64] = e4m3(gamma*q), col 64 = 1.0 (exact)
        # padded to 80B/chunk: DoubleRow ldweights needs k-tile step %16==0
        vhat8 = sing.tile([P, nch, 80], dt.float8e4)
        nc.gpsimd.memset(vhat8[:, :, C : C + 1], 1.0)
        # w[p, k, :] = (1+gamma)*q - fp32(vhat8): exact-diagonal residual
        w_sb = sing.tile([P, nch, C], dt.float32)
        idh = sing.tile([P, P], dt.float16)
        make_identity(nc, idh)
        # qT1 = [qT; 1] (lhsT), qTt = [A*qT; B - A*|q|^2] (rhs)
        qT1 = sing.tile([C + 1, ntok], dt.float16)
        qTt = sing.tile([C + 1, ntok], dt.float16)
        nc.gpsimd.memset(qT1[C : C + 1, :], 1.0)

        # permuted token order: partition p holds tokens 32p..32p+31 so each
        # partition reads one contiguous 8KB run of x. The whole kernel is
        # consistent in this order, incl. output writeback.
        xg = x.ap().rearrange("(p k) c -> p k c", k=nch)
        og = o.ap().rearrange("(p k) c -> p k c", k=nch)
        sqp = ctx.enter_context(tc.tile_pool(name="sqp", bufs=2))
        aux = ctx.enter_context(tc.tile_pool(name="aux", bufs=1, space="PSUM"))
        spool = ctx.enter_context(tc.tile_pool(name="spool", bufs=s_bufs, space="PSUM"))
        gpool = ctx.enter_context(tc.tile_pool(name="gpool", bufs=1, space="PSUM"))
        zpool = ctx.enter_context(tc.tile_pool(name="zpool", bufs=z_bufs))
        gsb = ctx.enter_context(tc.tile_pool(name="gsb", bufs=2))
        esb = ctx.enter_context(tc.tile_pool(name="esb", bufs=4))

        # Warm-up transposes: the PE's HAM throttle ramps on a free-running
        # activity window; burning the otherwise-idle framework-startup gap
        # gets the first real matmuls to 2.4 GHz instead of the cold clock.
        for _ in range(12):
            wt = aux.tile([P, P], dt.float32, tag="aux")
            nc.tensor.transpose(out=wt, in_=ident, identity=ident)

        # Deferred low-priority DVE work (w residuals, epilogue chunks) is
        # drained one thunk per key-pair so the in-order DVE queue never
        # develops a multi-us burst that would delay exp tiles -> stall the
        # PE's PSUM recycling -> drop the PE out of its high-clock state.
        thunks = []

        def emit_group(gi):
            """Load + preprocess chunks [4gi, 4gi+4)."""
            ks = slice(gi * pgrp, (gi + 1) * pgrp)
            eng = nc.sync if gi % 2 == 0 else nc.gpsimd
            eng.dma_start(out=q_sb[:, ks, 0:C], in_=xg[:, ks, :])
            sq = sqp.tile([P, pgrp, C], dt.float32)
            nc.vector.tensor_mul(sq, q_sb[:, ks, 0:C], q_sb[:, ks, 0:C])
            rg = sqp.tile([P, pgrp], dt.float32, tag="rg")
            nc.vector.reduce_sum(out=rg, in_=sq, axis=mybir.AxisListType.X)
            nc.vector.tensor_scalar(out=q_sb[:, ks, C : C + 1],
                                    in0=rg.unsqueeze(2), scalar1=-1.0,
                                    scalar2=BSH / AS, op0=ALU.mult, op1=ALU.add)
            qf = sqp.tile([P, pgrp, C + 1], dt.float16, tag="qf")
            nc.vector.tensor_copy(out=qf, in_=q_sb[:, ks, :])
            tp = aux.tile([C + 1, pgrp * P], dt.float16, tag="aux")
            for j in range(pgrp):
                nc.tensor.transpose(out=tp[:, j * P : (j + 1) * P],
                                    in_=qf[:, j, :], identity=idh)
            sl = slice(gi * pgrp * P, (gi + 1) * pgrp * P)
            # rhs gets the A scale (shift col was pre-divided by A)
            nc.vector.tensor_scalar_mul(qTt[:, sl], tp, AS)
            nc.scalar.copy(out=qT1[0:C, sl], in_=tp[0:C, :])
            nc.vector.tensor_scalar_mul(vhat8[:, ks, 0:C], q_sb[:, ks, 0:C], gam)

            def wprep(ks=ks):
                # exact-diagonal residual w; only needed by the epilogue
                v8f = sqp.tile([P, pgrp, C], dt.float32, tag="v8f")
                nc.vector.tensor_scalar_mul(v8f, vhat8[:, ks, 0:C], 1.0)
                nc.vector.tensor_scalar_mul(w_sb[:, ks, :], q_sb[:, ks, 0:C], gam1)
                nc.vector.tensor_sub(w_sb[:, ks, :], w_sb[:, ks, :], v8f)

            thunks.append(wprep)

        emitted = 0  # groups emitted so far

        def need_groups(n):
            nonlocal emitted
            while emitted < min(n, ngrp):
                emit_group(emitted)
                emitted += 1

        def epi_chunk(js, gs, e, last):
            ch = js * ech + e
            gtp = aux.tile([P, C + 1], dt.float32, tag="aux")
            nc.tensor.transpose(out=gtp, in_=gs[:, e * P : (e + 1) * P],
                                identity=ident[0 : C + 1, 0 : C + 1])
            rec = esb.tile([P, 1], dt.float32)
            nc.vector.reciprocal(out=rec, in_=gtp[:, C : C + 1])
            oc = esb.tile([P, C], dt.float32, tag="oc")
            if last:
                # tail only: ACT is idle once the exps are done
                nc.scalar.activation(out=oc, in_=gtp[:, 0:C], func=AF.Copy,
                                     scale=rec)
            else:
                # keep ACT a pure exp stream: its queue latency gates the
                # PE's PSUM recycling
                nc.vector.tensor_scalar_mul(oc, gtp[:, 0:C], rec)
            nc.vector.tensor_add(oc, oc, w_sb[:, ch, :])
            (nc.sync if e % 2 == 0 else nc.gpsimd).dma_start(
                out=og[:, ch, :], in_=oc)

        for js in range(nsup):
            gt = gpool.tile([C + 1, supw], dt.float32)
            ndve = dve_js0 if js == 0 else dve_js

            def g_mms(zt, gp, gt=gt):
                for i in range(nmm):
                    nc.tensor.matmul(gt[:, i * mw : (i + 1) * mw],
                                     vhat8[:, 2 * gp : 2 * gp + 2, 0 : C + 1],
                                     zt[:, :, i * mw : (i + 1) * mw],
                                     start=(gp == 0), stop=(gp == npair - 1),
                                     perf_mode=PM.DoubleRow)

            pending = []
            for gp in range(npair):
                need_groups(max((js * supw + supw - 1) // (pgrp * P) + 1,
                                (2 * gp + 1) // pgrp + 2))
                zt = zpool.tile([P, 2, supw], dt.float8e4)
                # one PSUM tile holds both chunks of the pair so a single
                # ap-1024 instruction converts it (amortizes engine init)
                st = spool.tile([P, 2, supw], dt.float32)
                for h in range(2):
                    m = 2 * gp + h
                    lhs = qT1[:, m * P : (m + 1) * P]
                    for i in range(nmm):
                        nc.tensor.matmul(st[:, h, i * mw : (i + 1) * mw], lhs,
                                         qTt[:, js * supw + i * mw : js * supw + (i + 1) * mw],
                                         start=True, stop=True)
                if ndve and (gp % 3 == 2 or gp == 7):
                    # Schraudolph: clamp to [0, 118] and truncate to int8;
                    # the bits ARE the e4m3 value.
                    nc.vector.tensor_scalar(out=zt.bitcast(dt.int8),
                                            in0=st, scalar1=0.0,
                                            scalar2=CLIP_HI,
                                            op0=ALU.max, op1=ALU.min)
                else:
                    nc.scalar.activation(out=zt, in_=st, func=AF.Exp,
                                         scale=EXP_SCALE, bias=ebias)
                pending.append((zt, gp))
                if len(pending) > glag:
                    g_mms(*pending.pop(0))
                if thunks and gp % 2 == 0:
                    thunks.pop(0)()
            for pz in pending:
                g_mms(*pz)
            last = js == nsup - 1
            gs = gsb.tile([C + 1, supw], dt.float32)
            # gt must be free before the next super's first G matmul: copy it
            # out immediately; chunk math is deferred into the next super.
            (nc.scalar.copy if last else nc.vector.tensor_copy)(out=gs, in_=gt)
            for e in range(ech):
                if last:
                    epi_chunk(js, gs, e, last)
                else:
                    thunks.append(lambda js=js, gs=gs, e=e: epi_chunk(js, gs, e, False))
        while thunks:
            thunks.pop(0)()

    nc.compile()
    return nc


_CACHE = {}


def _get_nc(**kw):
    key = tuple(sorted(kw.items()))
    if key not in _CACHE:
        _CACHE[key] = build(**kw)
    return _CACHE[key]


def kernel(x: np.ndarray, gamma: np.ndarray) -> np.ndarray:
    """Full-input entry point: x (8,16,16,16,64) f32, gamma (1,) f32."""
    if LDW_OPT:
        _patch_ldw_opt()
    from concourse.bass_utils import run_bass_kernel_spmd

    Bf, D, H, W, Cf = x.shape
    ntok = D * H * W
    xf = np.ascontiguousarray(np.asarray(x, dtype=np.float32).reshape(Bf, ntok, Cf))
    gf = np.ascontiguousarray(np.asarray(gamma, dtype=np.float32).reshape(1))
    nc = _get_nc(ntok=ntok)
    in_maps = [{"x": xf[b], "gamma": gf} for b in range(Bf)]
    res = run_bass_kernel_spmd(nc, in_maps, core_ids=list(range(Bf)))
    out = np.stack([res.results[b]["out"] for b in range(Bf)], axis=0)
    return out.reshape(x.shape).astype(x.dtype, copy=False)


# revision 21
# speedup vs baseline: 1.4635x; 1.4635x over previous
"""Channel self-attention kernel for TRN2, data-parallel over batch on 8 cores.

Math per batch element (N=4096 tokens, C=64 channels):
    q = x.reshape(N, C);  S = q @ q.T  (symmetric)
    attn = softmax(S, axis=-1);  out = gamma * (attn @ q) + x

Implementation notes (v2, fp8 pipeline):
  - The S matmul computes Y = A*(S - 2t_n) + B directly: lhsT = [qT; 1]
    (65 x 128 fp16), rhs = [A*qT; B - A*|q|^2] (65 x 512 fp16), where
    A = 8/ln2 and B = 96.  Y is the exp argument expressed in fp8e4m3
    *bit units*: exp(S - 2t_n) * 2^{(B-56)/8} == e4m3_from_bits(Y).
    The per-column shift -2t_n keeps all exponents <= ~0 (Cauchy-Schwarz:
    S_mn - 2t_n = q_m.q_n - |q_n|^2), so Z fits fp8 range; any error in
    the shift row is column-uniform and cancels in the softmax divide.
  - Z tiles are produced in fp8e4m3 two ways, split across engines:
      ACT: activation(Exp, scale=1/A, bias=-56.5/A) -> true exp, fp8 out.
      DVE: tensor_scalar(max 0, min 118.49) -> int8 = Schraudolph bit-trick
           (linear-in-mantissa exp approx, ~4% on off-diagonal weights).
    Both land on the same 2^{(B-56)/8} scale (the 0.5 in the ACT bias
    compensates int8 truncation); tiles can be mixed freely per column.
  - The attend matmul runs in fp8 DoubleRow mode: two 128-key chunks per
    pass (contraction 256).  On HW this streams both k-tiles (~2 cycles
    per output column) so it matches bf16 MAC rate, but it halves the
    instruction/ldweights count, which measures faster.  lhsT = vhat8
    [128, 2, 65] = e4m3(gamma*q) plus an exact-1.0 denominator row; rhs =
    Z8 [128, 2, supw].  Key pipeline facts (measured): the PE's HAM
    throttle needs a gap-free instruction stream to hold 2.4 GHz, so S
    tiles are 3-deep in PSUM, the attend matmul lags its exps by 3 pairs,
    and DVE/ACT side work is drained one small thunk per pair.
  - fp8 quantization of gamma*q is corrected exactly on the diagonal:
    out = G/d + ((1+gamma)*q - fp32(vhat8)), which equals
    sum_m (Z_mn/d)(v8[m]-v8[n]) + (1+gamma)q -- fp8 noise only multiplies
    off-diagonal attention mass (~1e-4 relative overall).
  - Matmuls for S run in fp16 at 1 cycle/row; prologue is pipelined in
    groups of 4 row-chunks as before.
"""
import sys
if "/opt/trn_rl_repo" not in sys.path:
    sys.path.insert(0, "/opt/trn_rl_repo")

from contextlib import ExitStack
import math

import numpy as np

import concourse.bass as bass
import concourse.mybir as mybir
import concourse.tile as tile
from concourse import bacc
from concourse.masks import make_identity

P = 128          # partitions
C = 64           # channels (head dim)
B = 8            # batch = number of cores

dt = mybir.dt
AF = mybir.ActivationFunctionType
ALU = mybir.AluOpType
PM = mybir.MatmulPerfMode

AS = 8.0 / math.log(2.0)     # Schraudolph slope: e4m3 bits per e-fold
BSH = 96.0                   # bit offset in the shift row (e4m3 2^5)
EXP_SCALE = 1.0 / AS
EXP_BIAS = -56.5 / AS        # 56 = e4m3 bias<<3; +0.5 matches int8 trunc
CLIP_HI = 118.49             # max bits kept finite in both e4m3 flavors

LDW_OPT = False  # ldw-opt incompatible with explicit 16-bit LDWEIGHTS


def _patch_ldw_opt():
    import concourse.bass_utils as bu
    if getattr(bu, "_ldw_opt_patch", False):
        return
    orig = bu.bir_verify_and_optimise

    def patched(*a, **kw):
        orig_run = bu.run_command

        def run2(argv, **k):
            argv = ["--enable-ldw-opt=true" if x == "--enable-ldw-opt=false" else x
                    for x in argv]
            return orig_run(argv, **k)

        bu.run_command = run2
        try:
            return orig(*a, **kw)
        finally:
            bu.run_command = orig_run

    bu.bir_verify_and_optimise = patched
    bu._ldw_opt_patch = True


def build(ntok=4096, supw=512, z_bufs=6, s_bufs=3, pgrp=4, dve_js0=0, dve_js=1, glag=3):
    """Build the per-core Bass module.

    dve_js0/dve_js: of the 2 chunks in each key pair, how many go to the
    DVE Schraudolph path during super 0 / later supers (0..2)."""
    nch = ntok // P           # query/key chunks of 128
    npair = nch // 2          # fp8 DoubleRow key-chunk pairs
    nsup = ntok // supw       # outer n-blocks
    mw = min(512, supw)       # matmul moving width
    nmm = supw // mw          # matmuls per n-super
    ech = supw // P           # epilogue 128-chunks per n-super
    pgrp = min(pgrp, nch)     # prologue chunks per group
    ngrp = nch // pgrp

    nc = bacc.Bacc("TRN2", target_bir_lowering=False, debug=False,
                   enable_asserts=False)
    x = nc.dram_tensor("x", [ntok, C], dt.float32, kind="ExternalInput")
    g = nc.dram_tensor("gamma", [1], dt.float32, kind="ExternalInput")
    o = nc.dram_tensor("out", [ntok, C], dt.float32, kind="ExternalOutput")

    with tile.TileContext(nc) as tc, ExitStack() as ctx:
        sing = ctx.enter_context(tc.tile_pool(name="sing", bufs=1))

        ident = sing.tile([P, P], dt.float32)
        make_identity(nc, ident)
        gam = sing.tile([P, 1], dt.float32)
        nc.sync.dma_start(out=gam, in_=g.ap().to_broadcast((P, 1)))
        gam1 = sing.tile([P, 1], dt.float32)
        nc.vector.tensor_scalar_add(gam1, gam, 1.0)
        ebias = sing.tile([P, 1], dt.float32)
        nc.gpsimd.memset(ebias, EXP_BIAS)

        # q_sb[p, k, 0:64] = x[token 32p+k, :]; col 64 = B/A - |q|^2
        q_sb = sing.tile([P, nch, C + 1], dt.float32)
        # vhat8[p, k, 0:64] = e4m3(gamma*q), col 64 = 1.0 (exact)
        # padded to 80B/chunk: DoubleRow ldweights needs k-tile step %16==0
        vhat8 = sing.tile([P, nch, 80], dt.float8e4)
        nc.gpsimd.memset(vhat8[:, :, C : C + 1], 1.0)
        # w[p, k, :] = (1+gamma)*q - fp32(vhat8): exact-diagonal residual
        w_sb = sing.tile([P, nch, C], dt.float32)
        idh = sing.tile([P, P], dt.float16)
        make_identity(nc, idh)
        # qT1 = [qT; 1] (lhsT), qTt = [A*qT; B - A*|q|^2] (rhs)
        qT1 = sing.tile([C + 1, ntok], dt.float16)
        qTt = sing.tile([C + 1, ntok], dt.float16)
        nc.gpsimd.memset(qT1[C : C + 1, :], 1.0)

        # permuted token order: partition p holds tokens 32p..32p+31 so each
        # partition reads one contiguous 8KB run of x. The whole kernel is
        # consistent in this order, incl. output writeback.
        xg = x.ap().rearrange("(p k) c -> p k c", k=nch)
        og = o.ap().rearrange("(p k) c -> p k c", k=nch)
        sqp = ctx.enter_context(tc.tile_pool(name="sqp", bufs=2))
        aux = ctx.enter_context(tc.tile_pool(name="aux", bufs=1, space="PSUM"))
        spool = ctx.enter_context(tc.tile_pool(name="spool", bufs=s_bufs, space="PSUM"))
        gpool = ctx.enter_context(tc.tile_pool(name="gpool", bufs=1, space="PSUM"))
        zpool = ctx.enter_context(tc.tile_pool(name="zpool", bufs=z_bufs))
        gsb = ctx.enter_context(tc.tile_pool(name="gsb", bufs=2))
        esb = ctx.enter_context(tc.tile_pool(name="esb", bufs=4))

        # Deferred low-priority DVE work (w residuals, epilogue chunks) is
        # drained one thunk per key-pair so the in-order DVE queue never
        # develops a multi-us burst that would delay exp tiles -> stall the
        # PE's PSUM recycling -> drop the PE out of its high-clock state.
        thunks = []

        def emit_group(gi):
            """Load + preprocess chunks [4gi, 4gi+4)."""
            ks = slice(gi * pgrp, (gi + 1) * pgrp)
            eng = nc.sync if gi % 2 == 0 else nc.gpsimd
            eng.dma_start(out=q_sb[:, ks, 0:C], in_=xg[:, ks, :])
            sq = sqp.tile([P, pgrp, C], dt.float32)
            nc.vector.tensor_mul(sq, q_sb[:, ks, 0:C], q_sb[:, ks, 0:C])
            rg = sqp.tile([P, pgrp], dt.float32, tag="rg")
            nc.vector.reduce_sum(out=rg, in_=sq, axis=mybir.AxisListType.X)
            nc.vector.tensor_scalar(out=q_sb[:, ks, C : C + 1],
                                    in0=rg.unsqueeze(2), scalar1=-1.0,
                                    scalar2=BSH / AS, op0=ALU.mult, op1=ALU.add)
            qf = sqp.tile([P, pgrp, C + 1], dt.float16, tag="qf")
            nc.vector.tensor_copy(out=qf, in_=q_sb[:, ks, :])
            tp = aux.tile([C + 1, pgrp * P], dt.float16, tag="aux")
            for j in range(pgrp):
                nc.tensor.transpose(out=tp[:, j * P : (j + 1) * P],
                                    in_=qf[:, j, :], identity=idh)
            sl = slice(gi * pgrp * P, (gi + 1) * pgrp * P)
            # rhs gets the A scale (shift col was pre-divided by A)
            nc.vector.tensor_scalar_mul(qTt[:, sl], tp, AS)
            nc.scalar.copy(out=qT1[0:C, sl], in_=tp[0:C, :])
            nc.vector.tensor_scalar_mul(vhat8[:, ks, 0:C], q_sb[:, ks, 0:C], gam)

            def wprep(ks=ks):
                # exact-diagonal residual w; only needed by the epilogue
                v8f = sqp.tile([P, pgrp, C], dt.float32, tag="v8f")
                nc.vector.tensor_scalar_mul(v8f, vhat8[:, ks, 0:C], 1.0)
                nc.vector.tensor_scalar_mul(w_sb[:, ks, :], q_sb[:, ks, 0:C], gam1)
                nc.vector.tensor_sub(w_sb[:, ks, :], w_sb[:, ks, :], v8f)

            thunks.append(wprep)

        emitted = 0  # groups emitted so far

        def need_groups(n):
            nonlocal emitted
            while emitted < min(n, ngrp):
                emit_group(emitted)
                emitted += 1

        def epi_chunk(js, gs, e, last):
            ch = js * ech + e
            gtp = aux.tile([P, C + 1], dt.float32, tag="aux")
            nc.tensor.transpose(out=gtp, in_=gs[:, e * P : (e + 1) * P],
                                identity=ident[0 : C + 1, 0 : C + 1])
            rec = esb.tile([P, 1], dt.float32)
            nc.vector.reciprocal(out=rec, in_=gtp[:, C : C + 1])
            oc = esb.tile([P, C], dt.float32, tag="oc")
            if last:
                # tail only: ACT is idle once the exps are done
                nc.scalar.activation(out=oc, in_=gtp[:, 0:C], func=AF.Copy,
                                     scale=rec)
            else:
                # keep ACT a pure exp stream: its queue latency gates the
                # PE's PSUM recycling
                nc.vector.tensor_scalar_mul(oc, gtp[:, 0:C], rec)
            nc.vector.tensor_add(oc, oc, w_sb[:, ch, :])
            (nc.sync if e % 2 == 0 else nc.gpsimd).dma_start(
                out=og[:, ch, :], in_=oc)

        for js in range(nsup):
            gt = gpool.tile([C + 1, supw], dt.float32)
            ndve = dve_js0 if js == 0 else dve_js

            def g_mms(zt, gp, gt=gt):
                for i in range(nmm):
                    nc.tensor.matmul(gt[:, i * mw : (i + 1) * mw],
                                     vhat8[:, 2 * gp : 2 * gp + 2, 0 : C + 1],
                                     zt[:, :, i * mw : (i + 1) * mw],
                                     start=(gp == 0), stop=(gp == npair - 1),
                                     perf_mode=PM.DoubleRow)

            pending = []
            for gp in range(npair):
                need_groups(max((js * supw + supw - 1) // (pgrp * P) + 1,
                                (2 * gp + 1) // pgrp + 2))
                zt = zpool.tile([P, 2, supw], dt.float8e4)
                # one PSUM tile holds both chunks of the pair so a single
                # ap-1024 instruction converts it (amortizes engine init)
                st = spool.tile([P, 2, supw], dt.float32)
                for h in range(2):
                    m = 2 * gp + h
                    lhs = qT1[:, m * P : (m + 1) * P]
                    for i in range(nmm):
                        nc.tensor.matmul(st[:, h, i * mw : (i + 1) * mw], lhs,
                                         qTt[:, js * supw + i * mw : js * supw + (i + 1) * mw],
                                         start=True, stop=True)
                if ndve and (gp % 3 == 2 or gp == 7):
                    # Schraudolph: clamp to [0, 118] and truncate to int8;
                    # the bits ARE the e4m3 value.
                    nc.vector.tensor_scalar(out=zt.bitcast(dt.int8),
                                            in0=st, scalar1=0.0,
                                            scalar2=CLIP_HI,
                                            op0=ALU.max, op1=ALU.min)
                else:
                    nc.scalar.activation(out=zt, in_=st, func=AF.Exp,
                                         scale=EXP_SCALE, bias=ebias)
                pending.append((zt, gp))
                if len(pending) > glag:
                    g_mms(*pending.pop(0))
                if thunks and gp % 2 == 0:
                    thunks.pop(0)()
            for pz in pending:
                g_mms(*pz)
            last = js == nsup - 1
            gs = gsb.tile([C + 1, supw], dt.float32)
            # gt must be free before the next super's first G matmul: copy it
            # out immediately; chunk math is deferred into the next super.
            (nc.scalar.copy if last else nc.vector.tensor_copy)(out=gs, in_=gt)
            for e in range(ech):
                if last:
                    epi_chunk(js, gs, e, last)
                else:
                    thunks.append(lambda js=js, gs=gs, e=e: epi_chunk(js, gs, e, False))
        while thunks:
            thunks.pop(0)()

    nc.compile()
    return nc


_CACHE = {}


def _get_nc(**kw):
    key = tuple(sorted(kw.items()))
    if key not in _CACHE:
        _CACHE[key] = build(**kw)
    return _CACHE[key]


def kernel(x: np.ndarray, gamma: np.ndarray) -> np.ndarray:
    """Full-input entry point: x (8,16,16,16,64) f32, gamma (1,) f32."""
    if LDW_OPT:
        _patch_ldw_opt()
    from concourse.bass_utils import run_bass_kernel_spmd

    Bf, D, H, W, Cf = x.shape
    ntok = D * H * W
    xf = np.ascontiguousarray(np.asarray(x, dtype=np.float32).reshape(Bf, ntok, Cf))
    gf = np.ascontiguousarray(np.asarray(gamma, dtype=np.float32).reshape(1))
    nc = _get_nc(ntok=ntok)
    in_maps = [{"x": xf[b], "gamma": gf} for b in range(Bf)]
    res = run_bass_kernel_spmd(nc, in_maps, core_ids=list(range(Bf)))
    out = np.stack([res.results[b]["out"] for b in range(Bf)], axis=0)
    return out.reshape(x.shape).astype(x.dtype, copy=False)
